# revision 49
# baseline (speedup 1.0000x reference)
"""Block-sparse attention kernel for Trainium2 (8 NeuronCores).

Problem: B=2, S=2048, H=16, Dqk=Dv=64, 64x64 block mask (30% + forced diag),
AND causal. out = softmax(mask(QK^T/8)) @ V.

Strategy
--------
- Shard the 32 (batch, head) pairs across 8 cores, 4 heads per core.
- Each core gets its OWN Bass program with the sparse block schedule baked in
  from its heads' block masks (compiled at call time, run concurrently on the
  8 axon devices).
- Per head, scores are computed TRANSPOSED (S^T[k, q]) so that P^T = exp(S^T)
  lands in SBUF in exactly the layout PV needs (k on partitions):
    * host supplies Q^T and K^T as [64(d), 2048(s)] fp16, V as [128, 16*65]
      fp16 "v-pair" tiles [V[kb1]; V[kb2]] with a ones column (col 64).
    * k-blocks are paired greedily to maximize active-q overlap; a pair forms
      a 128-partition tile.
    * QK: matmul(lhsT=K^T pair [64,128], rhs=Q^T qb-run [64,64n]) -> PSUM.
    * exp: one ACT op per <=1024 PSUM columns (scale=1/8 fused), fp16 out.
    * fixups: causal-triangle multiplies on diagonal blocks plus memsets of
      dead halves inside mixed PV runs (split between DVE and Pool). A PV
      run whose columns all share one live-half pattern instead uses weight
      variants with the dead half zeroed ([V1;0] / [0;V2]) so the exp
      garbage there is multiplied by zero — all matmuls stay full-128
      contract (the HW mishandles back-to-back 64-contract weight loads).
    * PV: matmul(lhsT=[V|1] pair variant, rhs=P^T run) accumulating
      O^T[65, 512] per (head, q-bank) in a rotating 1-bank PSUM tile.
    * O^T (unnormalized, row 64 = softmax denominator l) is copied to SBUF
      as fp16 (DVE) and DMA'd out; the host divides and transposes.
- The PE instruction stream is SOFTWARE-PIPELINED: chunk i's PV matmuls are
  emitted after chunk (i+LOOKAHEAD)'s QK, so the PE never idles waiting for
  the exp/fixup of the current chunk (idle gaps re-engage the HAM clock gate
  and halve the PE clock).
- Consecutive matmuls sharing identical weights keep only the first
  InstLdweights (_strip_repeated_weights); LDWEIGHTS per small matmul is
  otherwise the dominant PE-queue cost on this walrus.
- Softmax uses no running max: inputs are N(0,1) so scores/8 stay in a range
  where exp() is safely finite in fp16 (exp(~5) ~ 150).
"""

import threading
from collections import deque
from contextlib import ExitStack

import numpy as np

import concourse.bass as bass
import concourse.tile as tile
from concourse import mybir
from concourse.bass_utils import run_bass_kernel_spmd
from concourse.vector_clock import ScopedClock

# ----------------------------------------------------------------------------
# Workaround: the installed walrus rejects instructions with more than one
# sync wait. Tile's kernel-tail drain attaches every outstanding clock sem to
# one Drain instruction; split them one wait per Drain.
# ----------------------------------------------------------------------------


def _split_drain_and_barrier(self, tick_clock, wait_clock):
    nc = self.nc
    drain_inst = nc.sync.drain()
    wait_clock.add_sem_waits(
        drain_inst.ins, ScopedClock({None: tick_clock.global_clock})
    )
    si = drain_inst.ins.sync_info
    waits = list(si.on_wait) if si is not None else []
    if len(waits) > 1:
        drain_inst.ins.sync_info = mybir.SyncInfo(
            on_wait=waits[:1], on_update=list(si.on_update)
        )
        for w in waits[1:]:
            d2 = nc.sync.drain()
            d2.ins.sync_info = mybir.SyncInfo(on_wait=[w], on_update=[])
    nc.all_engine_barrier()
    popped = nc._tile_sem_poison_stack.pop()
    assert popped is self._sem_poison
    nc.clear_and_free_semaphores(list(self.sems.allocated().values()))
    nc.all_engine_barrier()


tile.TileContext._drain_and_barrier = _split_drain_and_barrier


def _strip_repeated_weights(nc):
    """Post-scheduling peephole: the IR carries one explicit InstLdweights per
    matmul, so consecutive matmuls sharing identical weights reload them every
    time — and LDWEIGHTS dominates PE time for small-N matmuls. Drop every
    InstLdweights whose weights AP matches the previous one (with only
    matmuls/noops/events between): the first load of the chain services all
    of them. Dropped loads keep their sem waits/updates on a PE NoOp.
    """
    for fn in nc.m.functions:
        for bb in fn.blocks:
            out = []
            last_key = None
            changed = False
            for inst in bb.instructions:
                if isinstance(inst, mybir.InstLdweights):
                    w = inst.ins[0]
                    key = (
                        getattr(w, "memref", None),
                        w.offset,
                        str(w.ap),
                        str(getattr(w, "dtype", None)),
                        inst.tile_position,
                        inst.perf_mode,
                        inst.is_transpose,
                    )
                    if key == last_key:
                        si = inst.sync_info
                        if si is not None and (si.on_wait or si.on_update):
                            out.append(
                                mybir.InstNoOp(
                                    name=nc.get_next_instruction_name(),
                                    engine=inst.engine,
                                    sync_info=si,
                                    bass_nofuse=True,
                                )
                            )
                        changed = True
                        continue
                    last_key = key
                    out.append(inst)
                elif isinstance(
                    inst, (mybir.InstMatmult, mybir.InstNoOp, mybir.InstEventSemaphore)
                ):
                    out.append(inst)
                else:
                    if inst.engine == mybir.EngineType.PE:
                        last_key = None
                    out.append(inst)
            if changed:
                bb.instructions = out


def _split_multi_waits(nc):
    """Hoist extra sync waits onto same-engine NOPs (walrus: 1 wait/inst)."""
    for fn in nc.m.functions:
        for bb in fn.blocks:
            out = []
            changed = False
            for inst in bb.instructions:
                si = inst.sync_info
                if si is not None and len(si.on_wait) > 1:
                    waits = list(si.on_wait)
                    for w in waits[:-1]:
                        out.append(
                            mybir.InstNoOp(
                                name=nc.get_next_instruction_name(),
                                engine=inst.engine,
                                sync_info=mybir.SyncInfo(on_wait=[w], on_update=[]),
                                bass_nofuse=True,
                            )
                        )
                    inst.sync_info = mybir.SyncInfo(
                        on_wait=[waits[-1]], on_update=list(si.on_update)
                    )
                    changed = True
                out.append(inst)
            if changed:
                bb.instructions = out


# ---------------------------------------------------------------------------
# Problem constants (hardcoded per the task contract)
# ---------------------------------------------------------------------------
B, S, H, D = 2, 2048, 16, 64
NB = 32  # number of 64-wide blocks along S
N_CORES = 8
HPC = 4  # heads (flat b*H+h) per core
CHUNK = 16  # score col-blocks per PSUM chunk (16*64 = 1024 fp32 = 2 banks)
LOOKAHEAD = 4  # chunks of PE lookahead before a chunk's PV is emitted
F16 = mybir.dt.float16
F32 = mybir.dt.float32


def _match_pairs(mask):
    """Pair up the 32 k-blocks to maximize overlap of their active-q sets
    (greedy max-weight matching). Overlapping pairs make dense (dual) score
    columns, shrinking the union column count that drives QK/exp/PV work."""
    act = {
        kb: frozenset(qb for qb in range(kb, NB) if mask[qb, kb]) for kb in range(NB)
    }
    left = set(range(NB))
    pairs = []
    while left:
        best = None
        for i in left:
            for j in left:
                if j <= i:
                    continue
                sc = len(act[i] & act[j])
                if best is None or sc > best[0] or (sc == best[0] and (i, j) < best[1:]):
                    best = (sc, i, j)
        _, i, j = best
        pairs.append((i, j))
        left -= {i, j}
    pairs.sort()
    return pairs


def _head_schedule(mask, pairs, gap=0):
    """Columns of the S^T score layout for one head: g-major (q-bank), then
    pair, then qb. Interior qb-gaps of <= `gap` within a (g, t) sequence are
    bridged with fake columns (top=bot=False) so QK/PV runs merge into fewer,
    larger matmuls (each matmul pays a full LDWEIGHTS on this walrus)."""
    cols = []
    for g in range(NB // 8):
        for t, (kb1, kb2) in enumerate(pairs):
            seq = []
            for qb in range(8 * g, 8 * (g + 1)):
                top = qb >= kb1 and bool(mask[qb, kb1])
                bot = qb >= kb2 and bool(mask[qb, kb2])
                if top or bot:
                    seq.append((qb, top, bot))
            ext = []
            for qb, top, bot in seq:
                if ext and 1 < qb - ext[-1][0] <= gap + 1:
                    for fqb in range(ext[-1][0] + 1, qb):
                        ext.append((fqb, False, False))
                ext.append((qb, top, bot))
            for qb, top, bot in ext:
                cols.append(
                    {
                        "t": t,
                        "qb": qb,
                        "top": top,
                        "bot": bot,
                        "kb1": kb1,
                        "kb2": kb2,
                        "g": g,
                    }
                )
    return cols


def _chunks_of(cols):
    """Cut cols into chunks of <= CHUNK, never crossing a q-bank (g) edge."""
    chunks = []
    cur = []
    for c in cols:
        if cur and (len(cur) >= CHUNK or cur[-1]["g"] != c["g"]):
            chunks.append(cur)
            cur = []
        cur.append(c)
    if cur:
        chunks.append(cur)
    return chunks


def _variant(c):
    if c["top"] and c["bot"]:
        return "F"
    return "T" if c["top"] else "B"


def _plan_pv(chunk):
    """PV matmul plan for one chunk: group columns by pair t, split into runs
    of consecutive (position, qb). A run whose live columns all share one
    variant (and has no fake columns) uses that variant's weights directly;
    mixed runs fall back to the F weights plus memsets of the dead halves.
    Returns (runs, need_top, need_bot); runs entries are (t, vi, i0, qb0, n).
    """
    L = len(chunk)
    need_top = [False] * L
    need_bot = [False] * L
    groups = {}
    for i, c in enumerate(chunk):
        groups.setdefault(c["t"], []).append((i, c))
    raw = []
    for t, lst in sorted(groups.items()):
        cur = [lst[0]]
        for item in lst[1:]:
            pi, pc = cur[-1]
            ci, cc = item
            if ci == pi + 1 and cc["qb"] == pc["qb"] + 1:
                cur.append(item)
            else:
                raw.append((t, cur))
                cur = [item]
        raw.append((t, cur))
    plan = []
    for t, lst in raw:
        live = [_variant(c) for _, c in lst if c["top"] or c["bot"]]
        if not live:
            continue  # all-fake run: contributes nothing, PV skips it
        if len(set(live)) == 1 and len(live) == len(lst):
            vi = {"F": 0, "T": 1, "B": 2}[live[0]]
        else:
            vi = 0
            for i, c in lst:
                if not c["top"]:
                    need_top[i] = True
                if not c["bot"]:
                    need_bot[i] = True
        plan.append((t, vi, lst[0][0], lst[0][1]["qb"], len(lst)))
    # PSUM accumulation order is irrelevant; sort so same-(t, variant) runs
    # are adjacent and share one LDWEIGHTS.
    plan.sort(key=lambda p: (p[0], p[1]))
    return plan, need_top, need_bot


def build_program(schedules):
    """Build the Bass program for one core.

    schedules: list of HPC dicts {"pairs": [(kb1, kb2)]*16, "cols": [...]}.
    """
    nc = bass.Bass()
    qt = nc.declare_dram_parameter("qt", [HPC, 64, S], F16, isOutput=False)
    kt = nc.declare_dram_parameter("kt", [HPC, 64, S], F16, isOutput=False)
    # va: per pair t, three 65-col weight variants (F=[V1;V2], T=[V1;0],
    # B=[0;V2]). A PV run whose top/bottom half-block is inactive uses the
    # T/B variant: the dead half of P^T is multiplied by zero weights, so it
    # needs no memset — and all PVs stay full-128-contract (HW mishandles
    # back-to-back 64-contract weight loads). Deriving the variants on-chip
    # instead of via DMA was tried and LOST (~10us): the copies/memsets
    # contend with fixups on DVE/Pool and delay the PV stream.
    va = nc.declare_dram_parameter("va", [HPC, 128, 48 * 65], F16, isOutput=False)
    tri = nc.declare_dram_parameter("tri", [128, 64], F16, isOutput=False)
    ot = nc.declare_dram_parameter("ot", [HPC, 65, S], F16, isOutput=True)

    with tile.TileContext(nc) as tc, ExitStack() as ctx:
        const = ctx.enter_context(tc.tile_pool(name="const", bufs=1))
        # One input pool per head: readers of a pool wait on its last DMA, so
        # sharing one pool would gate head 0's first QK on head 3's inputs.
        hpools = [
            ctx.enter_context(tc.tile_pool(name=f"in{s}", bufs=1)) for s in range(HPC)
        ]
        pts = ctx.enter_context(tc.tile_pool(name="pts", bufs=LOOKAHEAD + 1))
        outp = ctx.enter_context(tc.tile_pool(name="outp", bufs=3))
        psS = ctx.enter_context(tc.tile_pool(name="psS", bufs=3, space="PSUM"))
        psO = ctx.enter_context(tc.tile_pool(name="psO", bufs=2, space="PSUM"))

        tri_t = const.tile([128, 64], F16, tag="tri")
        # tri rides the fast (scalar) DMA queue with head 0's tensors: on the
        # bulk queue its completion round-robins behind ~4 MB of input.
        nc.scalar.dma_start(out=tri_t[:], in_=tri[:])
        zeros = const.tile([128, 512], F16, tag="zeros")
        nc.vector.memset(zeros[:], 0.0)

        # PE warm-up: the HAM clock gate keeps a cold PE at 1.2 GHz; burn
        # ~14 us of dummy matmuls (covering the input DMA latency, which
        # gates the first QK) to reach 2.4 and keep the PE queue busy.
        wps = psS.tile([128, 64 * CHUNK], F32, tag="ps")
        for _ in range(24):
            nc.tensor.matmul(
                wps[:, 0:512],
                lhsT=zeros[:, 0:128],
                rhs=zeros[:, 0:512],
                start=True,
                stop=True,
            )

        qts, kts, vas = [], [], []
        for s in range(HPC):
            qs = hpools[s].tile([64, S], F16, tag=f"qt{s}")
            ks = hpools[s].tile([64, S], F16, tag=f"kt{s}")
            vs = hpools[s].tile([128, 48 * 65], F16, tag=f"va{s}")
            # Head 0's tensors go on their own DMA queue (scalar's): the DMA
            # engines round-robin all queued descriptors, so on a shared
            # queue even the first tensors complete only near the end of the
            # whole ~5 MB load, idling the PE for ~7 us right when the HAM
            # warm-up window expires.
            eng = nc.scalar if s == 0 else nc.sync
            eng.dma_start(out=qs[:], in_=qt[s])
            eng.dma_start(out=ks[:], in_=kt[s])
            eng.dma_start(out=vs[:], in_=va[s])
            qts.append(qs)
            kts.append(ks)
            vas.append(vs)

        # Prefetch the exp ACT table set (~2.7us) while input DMAs run.
        # Emitted AFTER the dma_start submissions: the table load would
        # otherwise sit ahead of head 0's DMA submits on the scalar queue
        # and delay the input gate by its ~2.7us.
        scr = const.tile([128, 64], F16, tag="scr")
        nc.scalar.activation(
            out=scr[:],
            in_=zeros[:, 0:64],
            func=mybir.ActivationFunctionType.Exp,
            scale=0.125,
        )

        # Global chunk list across heads so the PE pipeline never drains.
        items = []  # (s, chunk_cols)
        for s in range(HPC):
            for ch in _chunks_of(schedules[s]["cols"]):
                items.append((s, ch))

        # psO bank state: one open (s, g) accumulation at a time on the PV
        # side; finalize (copy + DMA) when the next group begins.
        state = {"key": None, "tile": None}

        def finalize_group():
            if state["key"] is None:
                return
            s, g = state["key"]
            o_sb = outp.tile([65, 512], F16, tag="o")
            nc.vector.tensor_copy(out=o_sb[:], in_=state["tile"][0:65, :])
            nc.sync.dma_start(out=ot[s][:, 512 * g : 512 * (g + 1)], in_=o_sb[:])
            state["key"] = None
            state["tile"] = None

        def emit_pv(s, chunk, pt, plan):
            g = chunk[0]["g"]
            if state["key"] != (s, g):
                finalize_group()
                oT = psO.tile([128, 512], F32, tag="psO")
                state["key"] = (s, g)
                state["tile"] = oT
                # start=True on the bank's first PV clears has_written for
                # the WHOLE bank (HW-probed), so every element's first writer
                # overwrites stale data and later writers accumulate — no
                # zero-open matmul needed.
                state["first"] = True
            oT = state["tile"]
            for t, vi, i0, qb0, n in plan:
                w = vas[s][:, 65 * (3 * t + vi) : 65 * (3 * t + vi + 1)]
                q0 = qb0 - 8 * g
                nc.tensor.matmul(
                    oT[0:65, 64 * q0 : 64 * (q0 + n)],
                    lhsT=w,
                    rhs=pt[:, 64 * i0 : 64 * (i0 + n)],
                    start=state.pop("first", False),
                    stop=True,
                    skip_group_check=True,
                )

        pending = deque()
        for idx, (s, chunk) in enumerate(items):
            L = len(chunk)
            ps = psS.tile([128, 64 * CHUNK], F32, tag="ps")

            # QK: lhsT = K^T pair (fixed per t), rhs = Q^T qb-run. Runs split
            # at t changes, qb gaps and PSUM bank (512-col) edges.
            runs = []
            cur = [(0, chunk[0])]
            for i, c in enumerate(chunk[1:], start=1):
                pi, pc = cur[-1]
                if (
                    c["t"] == pc["t"]
                    and c["qb"] == pc["qb"] + 1
                    and (i // 8) == (cur[0][0] // 8)
                ):
                    cur.append((i, c))
                else:
                    runs.append(cur)
                    cur = [(i, c)]
            runs.append(cur)
            for run in runs:
                i0, rc = run[0]
                n = len(run)
                nc.tensor.matmul(
                    ps[:, 64 * i0 : 64 * (i0 + n)],
                    lhsT=kts[s][:, 128 * rc["t"] : 128 * (rc["t"] + 1)],
                    rhs=qts[s][:, 64 * rc["qb"] : 64 * (rc["qb"] + n)],
                    start=True,
                    stop=True,
                )

            pt = pts.tile([128, 64 * CHUNK], F16, tag="pt")
            nc.scalar.activation(
                out=pt[:, : 64 * L],
                in_=ps[:, : 64 * L],
                func=mybir.ActivationFunctionType.Exp,
                scale=0.125,
            )

            # Fixups (alternate the engine per chunk to split the load
            # between DVE and Pool): causal triangles on diagonal blocks,
            # then batched memsets of dead halves inside mixed PV runs.
            plan, need_top, need_bot = _plan_pv(chunk)
            eng_tri = nc.vector if idx % 2 == 0 else nc.gpsimd
            eng_ms = nc.gpsimd if idx % 2 == 0 else nc.vector
            for i, c in enumerate(chunk):
                if c["top"] and c["qb"] == c["kb1"]:
                    eng_tri.tensor_mul(
                        pt[0:64, 64 * i : 64 * (i + 1)],
                        pt[0:64, 64 * i : 64 * (i + 1)],
                        tri_t[0:64],
                    )
                if c["bot"] and c["qb"] == c["kb2"]:
                    eng_tri.tensor_mul(
                        pt[64:128, 64 * i : 64 * (i + 1)],
                        pt[64:128, 64 * i : 64 * (i + 1)],
                        tri_t[64:128],
                    )
            for half, need in ((slice(0, 64), need_top), (slice(64, 128), need_bot)):
                i = 0
                while i < L:
                    if need[i]:
                        j = i
                        while j + 1 < L and need[j + 1]:
                            j += 1
                        eng_ms.memset(pt[half, 64 * i : 64 * (j + 1)], 0.0)
                        i = j + 1
                    else:
                        i += 1

            pending.append((s, chunk, pt, plan))
            if len(pending) > LOOKAHEAD:
                emit_pv(*pending.popleft())
        while pending:
            emit_pv(*pending.popleft())
        finalize_group()

    _strip_repeated_weights(nc)
    _split_multi_waits(nc)
    return nc


def _prep_inputs(q, k, v, schedules):
    """Per-core input arrays keyed as the programs expect."""
    # flat head g = b*H + h
    qt_all = np.ascontiguousarray(
        q.transpose(0, 2, 3, 1).reshape(B * H, D, S).astype(np.float16)
    )
    kt_nat = k.transpose(0, 2, 3, 1).reshape(B * H, D, S).astype(np.float16)
    kt_nat = kt_nat.reshape(B * H, D, NB, 64)
    kt_all = np.empty_like(kt_nat)
    for g in range(B * H):
        order = [kb for p in schedules[g]["pairs"] for kb in p]
        kt_all[g] = kt_nat[g][:, order, :]
    kt_all = np.ascontiguousarray(kt_all.reshape(B * H, D, S))
    v_aug = np.concatenate([v, np.ones((B, S, H, 1), v.dtype)], axis=3)  # [B,S,H,65]
    vb_all = v_aug.transpose(0, 2, 1, 3).reshape(B * H, NB, 64, 65)  # [g, kb, 64, 65]
    # va[g]: per pair t, three variants (F=[V1;V2], T=[V1;0], B=[0;V2]);
    # rows 0:64 = V[kb1] block, rows 64:128 = V[kb2].
    va_all = np.zeros((B * H, 128, 48 * 65), np.float16)
    for g in range(B * H):
        for t, (kb1, kb2) in enumerate(schedules[g]["pairs"]):
            va_all[g, 0:64, 195 * t : 195 * t + 65] = vb_all[g, kb1]
            va_all[g, 64:128, 195 * t : 195 * t + 65] = vb_all[g, kb2]
            va_all[g, 0:64, 195 * t + 65 : 195 * t + 130] = vb_all[g, kb1]
            va_all[g, 64:128, 195 * t + 130 : 195 * t + 195] = vb_all[g, kb2]
    # tri[kl, ql] = 1 where kl <= ql (allowed), both halves
    triu = np.triu(np.ones((64, 64), np.float16))
    tri_full = np.ascontiguousarray(np.concatenate([triu, triu], axis=0))
    in_maps = []
    for c in range(N_CORES):
        sl = slice(HPC * c, HPC * (c + 1))
        in_maps.append(
            {
                "qt": qt_all[sl],
                "kt": kt_all[sl],
                "va": va_all[sl],
                "tri": tri_full,
            }
        )
    return in_maps


def _schedules(block_mask):
    """Per flat head: greedy k-block pairing + column schedule."""
    masks_all = np.asarray(block_mask).reshape(B * H, NB, NB)
    scheds = []
    for g in range(B * H):
        pairs = _match_pairs(masks_all[g])
        scheds.append({"pairs": pairs, "cols": _head_schedule(masks_all[g], pairs)})
    return scheds


_PROG_CACHE = {}


def _get_programs(block_mask, schedules):
    key = np.asarray(block_mask).tobytes()
    if key not in _PROG_CACHE:
        _PROG_CACHE[key] = [
            build_program(schedules[HPC * c : HPC * (c + 1)]) for c in range(N_CORES)
        ]
    return _PROG_CACHE[key]


def run_cores(ncs, in_maps, trace=False):
    """Run the 8 per-core programs concurrently on the 8 devices."""
    import jax

    devs = jax.devices()
    results = [None] * N_CORES
    errs = [None] * N_CORES

    def _run(c):
        try:
            with jax.default_device(devs[c]):
                r = run_bass_kernel_spmd(
                    ncs[c], [in_maps[c]], core_ids=[0], trace=trace and c == 0
                )
                results[c] = r
        except Exception as e:  # noqa: BLE001
            errs[c] = e

    threads = [threading.Thread(target=_run, args=(c,)) for c in range(N_CORES)]
    for t in threads:
        t.start()
    for t in threads:
        t.join()
    for c, e in enumerate(errs):
        if e is not None:
            raise RuntimeError(f"core {c} failed") from e
    return results


def kernel(q, k, v, block_mask):
    q = np.asarray(q, dtype=np.float32)
    k = np.asarray(k, dtype=np.float32)
    v = np.asarray(v, dtype=np.float32)
    block_mask = np.asarray(block_mask).astype(bool)

    schedules = _schedules(block_mask)
    in_maps = _prep_inputs(q, k, v, schedules)
    ncs = _get_programs(block_mask, schedules)
    results = run_cores(ncs, in_maps)

    out = np.empty((B, S, H, D), np.float32)
    for c in range(N_CORES):
        ot = results[c].results[0]["ot"]  # [HPC, 65, S] fp16
        for s in range(HPC):
            g = HPC * c + s
            b, h = divmod(g, H)
            o_un = ot[s, :D, :].astype(np.float32)  # [D, S] unnormalized
            l = ot[s, D, :].astype(np.float32)  # [S]
            out[b, :, h, :] = (o_un / l[None, :]).T
    return out


# revision 50
# speedup vs baseline: 1.0549x; 1.0549x over previous
"""Block-sparse attention kernel for Trainium2 (8 NeuronCores).

Problem: B=2, S=2048, H=16, Dqk=Dv=64, 64x64 block mask (30% + forced diag),
AND causal. out = softmax(mask(QK^T/8)) @ V.

Strategy
--------
- Shard the 32 (batch, head) pairs across 8 cores, 4 heads per core.
- Each core gets its OWN Bass program with the sparse block schedule baked in
  from its heads' block masks (compiled at call time, run concurrently on the
  8 axon devices).
- Per head, scores are computed TRANSPOSED (S^T[k, q]) so that P^T = exp(S^T)
  lands in SBUF in exactly the layout PV needs (k on partitions):
    * host supplies Q^T and K^T as [64(d), 2048(s)] fp16, V as [128, 16*65]
      fp16 "v-pair" tiles [V[kb1]; V[kb2]] with a ones column (col 64).
    * k-blocks are paired greedily to maximize active-q overlap; a pair forms
      a 128-partition tile.
    * QK: matmul(lhsT=K^T pair [64,128], rhs=Q^T qb-run [64,64n]) -> PSUM.
    * exp: one ACT op per <=1024 PSUM columns (scale=1/8 fused), fp16 out.
    * fixups: causal-triangle multiplies on diagonal blocks plus memsets of
      dead halves inside mixed PV runs (split between DVE and Pool). A PV
      run whose columns all share one live-half pattern instead uses weight
      variants with the dead half zeroed ([V1;0] / [0;V2]) so the exp
      garbage there is multiplied by zero — all matmuls stay full-128
      contract (the HW mishandles back-to-back 64-contract weight loads).
    * PV: matmul(lhsT=[V|1] pair variant, rhs=P^T run) accumulating
      O^T[65, 512] per (head, q-bank) in a rotating 1-bank PSUM tile.
    * O^T (unnormalized, row 64 = softmax denominator l) is copied to SBUF
      as fp16 (DVE) and DMA'd out; the host divides and transposes.
- The PE instruction stream is SOFTWARE-PIPELINED: chunk i's PV matmuls are
  emitted after chunk (i+LOOKAHEAD)'s QK, so the PE never idles waiting for
  the exp/fixup of the current chunk (idle gaps re-engage the HAM clock gate
  and halve the PE clock).
- Consecutive matmuls sharing identical weights keep only the first
  InstLdweights (_strip_repeated_weights); LDWEIGHTS per small matmul is
  otherwise the dominant PE-queue cost on this walrus.
- Softmax uses no running max: inputs are N(0,1) so scores/8 stay in a range
  where exp() is safely finite in fp16 (exp(~5) ~ 150).
"""

import threading
from collections import deque
from contextlib import ExitStack

import numpy as np

import concourse.bass as bass
import concourse.tile as tile
from concourse import mybir
from concourse.bass_utils import run_bass_kernel_spmd
from concourse.vector_clock import ScopedClock

# ----------------------------------------------------------------------------
# Workaround: the installed walrus rejects instructions with more than one
# sync wait. Tile's kernel-tail drain attaches every outstanding clock sem to
# one Drain instruction; split them one wait per Drain.
# ----------------------------------------------------------------------------


def _split_drain_and_barrier(self, tick_clock, wait_clock):
    nc = self.nc
    drain_inst = nc.sync.drain()
    wait_clock.add_sem_waits(
        drain_inst.ins, ScopedClock({None: tick_clock.global_clock})
    )
    si = drain_inst.ins.sync_info
    waits = list(si.on_wait) if si is not None else []
    if len(waits) > 1:
        drain_inst.ins.sync_info = mybir.SyncInfo(
            on_wait=waits[:1], on_update=list(si.on_update)
        )
        for w in waits[1:]:
            d2 = nc.sync.drain()
            d2.ins.sync_info = mybir.SyncInfo(on_wait=[w], on_update=[])
    nc.all_engine_barrier()
    popped = nc._tile_sem_poison_stack.pop()
    assert popped is self._sem_poison
    nc.clear_and_free_semaphores(list(self.sems.allocated().values()))
    nc.all_engine_barrier()


tile.TileContext._drain_and_barrier = _split_drain_and_barrier


def _strip_repeated_weights(nc):
    """Post-scheduling peephole: the IR carries one explicit InstLdweights per
    matmul, so consecutive matmuls sharing identical weights reload them every
    time — and LDWEIGHTS dominates PE time for small-N matmuls. Drop every
    InstLdweights whose weights AP matches the previous one (with only
    matmuls/noops/events between): the first load of the chain services all
    of them. Dropped loads keep their sem waits/updates on a PE NoOp.
    """
    for fn in nc.m.functions:
        for bb in fn.blocks:
            out = []
            last_key = None
            changed = False
            for inst in bb.instructions:
                if isinstance(inst, mybir.InstLdweights):
                    w = inst.ins[0]
                    key = (
                        getattr(w, "memref", None),
                        w.offset,
                        str(w.ap),
                        str(getattr(w, "dtype", None)),
                        inst.tile_position,
                        inst.perf_mode,
                        inst.is_transpose,
                    )
                    if key == last_key:
                        si = inst.sync_info
                        if si is not None and (si.on_wait or si.on_update):
                            out.append(
                                mybir.InstNoOp(
                                    name=nc.get_next_instruction_name(),
                                    engine=inst.engine,
                                    sync_info=si,
                                    bass_nofuse=True,
                                )
                            )
                        changed = True
                        continue
                    last_key = key
                    out.append(inst)
                elif isinstance(
                    inst, (mybir.InstMatmult, mybir.InstNoOp, mybir.InstEventSemaphore)
                ):
                    out.append(inst)
                else:
                    if inst.engine == mybir.EngineType.PE:
                        last_key = None
                    out.append(inst)
            if changed:
                bb.instructions = out


def _split_multi_waits(nc):
    """Hoist extra sync waits onto same-engine NOPs (walrus: 1 wait/inst)."""
    for fn in nc.m.functions:
        for bb in fn.blocks:
            out = []
            changed = False
            for inst in bb.instructions:
                si = inst.sync_info
                if si is not None and len(si.on_wait) > 1:
                    waits = list(si.on_wait)
                    for w in waits[:-1]:
                        out.append(
                            mybir.InstNoOp(
                                name=nc.get_next_instruction_name(),
                                engine=inst.engine,
                                sync_info=mybir.SyncInfo(on_wait=[w], on_update=[]),
                                bass_nofuse=True,
                            )
                        )
                    inst.sync_info = mybir.SyncInfo(
                        on_wait=[waits[-1]], on_update=list(si.on_update)
                    )
                    changed = True
                out.append(inst)
            if changed:
                bb.instructions = out


# ---------------------------------------------------------------------------
# Problem constants (hardcoded per the task contract)
# ---------------------------------------------------------------------------
B, S, H, D = 2, 2048, 16, 64
NB = 32  # number of 64-wide blocks along S
N_CORES = 8
HPC = 4  # heads (flat b*H+h) per core
CHUNK = 16  # score col-blocks per PSUM chunk (16*64 = 1024 fp32 = 2 banks)
LOOKAHEAD = 4  # chunks of PE lookahead before a chunk's PV is emitted
F16 = mybir.dt.float16
F32 = mybir.dt.float32


def _match_pairs(mask):
    """Pair up the 32 k-blocks to maximize overlap of their active-q sets
    (greedy max-weight matching). Overlapping pairs make dense (dual) score
    columns, shrinking the union column count that drives QK/exp/PV work."""
    act = {
        kb: frozenset(qb for qb in range(kb, NB) if mask[qb, kb]) for kb in range(NB)
    }
    left = set(range(NB))
    pairs = []
    while left:
        best = None
        for i in left:
            for j in left:
                if j <= i:
                    continue
                sc = len(act[i] & act[j])
                if best is None or sc > best[0] or (sc == best[0] and (i, j) < best[1:]):
                    best = (sc, i, j)
        _, i, j = best
        pairs.append((i, j))
        left -= {i, j}
    pairs.sort()
    return pairs


def _head_schedule(mask, pairs, gap=0):
    """Columns of the S^T score layout for one head: g-major (q-bank), then
    pair, then qb. Interior qb-gaps of <= `gap` within a (g, t) sequence are
    bridged with fake columns (top=bot=False) so QK/PV runs merge into fewer,
    larger matmuls (each matmul pays a full LDWEIGHTS on this walrus)."""
    cols = []
    for g in range(NB // 8):
        for t, (kb1, kb2) in enumerate(pairs):
            seq = []
            for qb in range(8 * g, 8 * (g + 1)):
                top = qb >= kb1 and bool(mask[qb, kb1])
                bot = qb >= kb2 and bool(mask[qb, kb2])
                if top or bot:
                    seq.append((qb, top, bot))
            ext = []
            for qb, top, bot in seq:
                if ext and 1 < qb - ext[-1][0] <= gap + 1:
                    for fqb in range(ext[-1][0] + 1, qb):
                        ext.append((fqb, False, False))
                ext.append((qb, top, bot))
            for qb, top, bot in ext:
                cols.append(
                    {
                        "t": t,
                        "qb": qb,
                        "top": top,
                        "bot": bot,
                        "kb1": kb1,
                        "kb2": kb2,
                        "g": g,
                    }
                )
    return cols


def _chunks_of(cols):
    """Cut cols into chunks of <= CHUNK, never crossing a q-bank (g) edge."""
    chunks = []
    cur = []
    for c in cols:
        if cur and (len(cur) >= CHUNK or cur[-1]["g"] != c["g"]):
            chunks.append(cur)
            cur = []
        cur.append(c)
    if cur:
        chunks.append(cur)
    return chunks


def _variant(c):
    if c["top"] and c["bot"]:
        return "F"
    return "T" if c["top"] else "B"


def _plan_pv(chunk):
    """PV matmul plan for one chunk: group columns by pair t, split into runs
    of consecutive (position, qb). A run whose live columns all share one
    variant (and has no fake columns) uses that variant's weights directly;
    mixed runs fall back to the F weights plus memsets of the dead halves.
    Returns (runs, need_top, need_bot); runs entries are (t, vi, i0, qb0, n).
    """
    L = len(chunk)
    need_top = [False] * L
    need_bot = [False] * L
    groups = {}
    for i, c in enumerate(chunk):
        groups.setdefault(c["t"], []).append((i, c))
    raw = []
    for t, lst in sorted(groups.items()):
        cur = [lst[0]]
        for item in lst[1:]:
            pi, pc = cur[-1]
            ci, cc = item
            if ci == pi + 1 and cc["qb"] == pc["qb"] + 1:
                cur.append(item)
            else:
                raw.append((t, cur))
                cur = [item]
        raw.append((t, cur))
    plan = []
    for t, lst in raw:
        live = [_variant(c) for _, c in lst if c["top"] or c["bot"]]
        if not live:
            continue  # all-fake run: contributes nothing, PV skips it
        if len(set(live)) == 1 and len(live) == len(lst):
            vi = {"F": 0, "T": 1, "B": 2}[live[0]]
        else:
            vi = 0
            for i, c in lst:
                if not c["top"]:
                    need_top[i] = True
                if not c["bot"]:
                    need_bot[i] = True
        plan.append((t, vi, lst[0][0], lst[0][1]["qb"], len(lst)))
    # PSUM accumulation order is irrelevant; sort so same-(t, variant) runs
    # are adjacent and share one LDWEIGHTS.
    plan.sort(key=lambda p: (p[0], p[1]))
    return plan, need_top, need_bot


def build_program(schedules):
    """Build the Bass program for one core.

    schedules: list of HPC dicts {"pairs": [(kb1, kb2)]*16, "cols": [...]}.
    """
    nc = bass.Bass()
    qt = nc.declare_dram_parameter("qt", [HPC, 64, S], F16, isOutput=False)
    kt = nc.declare_dram_parameter("kt", [HPC, 64, S], F16, isOutput=False)
    # va: per pair t, three 65-col weight variants (F=[V1;V2], T=[V1;0],
    # B=[0;V2]). A PV run whose top/bottom half-block is inactive uses the
    # T/B variant: the dead half of P^T is multiplied by zero weights, so it
    # needs no memset — and all PVs stay full-128-contract (HW mishandles
    # back-to-back 64-contract weight loads). Deriving the variants on-chip
    # instead of via DMA was tried and LOST (~10us): the copies/memsets
    # contend with fixups on DVE/Pool and delay the PV stream.
    va = nc.declare_dram_parameter("va", [HPC, 128, 48 * 65], F16, isOutput=False)
    tri = nc.declare_dram_parameter("tri", [128, 64], F16, isOutput=False)
    ot = nc.declare_dram_parameter("ot", [HPC, 65, S], F16, isOutput=True)

    with tile.TileContext(nc) as tc, ExitStack() as ctx:
        const = ctx.enter_context(tc.tile_pool(name="const", bufs=1))
        # One input pool per head: readers of a pool wait on its last DMA, so
        # sharing one pool would gate head 0's first QK on head 3's inputs.
        hpools = [
            ctx.enter_context(tc.tile_pool(name=f"in{s}", bufs=1)) for s in range(HPC)
        ]
        pts = ctx.enter_context(tc.tile_pool(name="pts", bufs=LOOKAHEAD + 1))
        outp = ctx.enter_context(tc.tile_pool(name="outp", bufs=3))
        psS = ctx.enter_context(tc.tile_pool(name="psS", bufs=3, space="PSUM"))
        psO = ctx.enter_context(tc.tile_pool(name="psO", bufs=2, space="PSUM"))

        tri_t = const.tile([128, 64], F16, tag="tri")
        # tri rides the fast (scalar) DMA queue with head 0's tensors: on the
        # bulk queue its completion round-robins behind ~4 MB of input.
        nc.scalar.dma_start(out=tri_t[:], in_=tri[:])
        zeros = const.tile([128, 512], F16, tag="zeros")
        nc.vector.memset(zeros[:], 0.0)

        # PE warm-up: the HAM clock gate keeps a cold PE at 1.2 GHz; burn
        # ~14 us of dummy matmuls (covering the input DMA latency, which
        # gates the first QK) to reach 2.4 and keep the PE queue busy.
        wps = psS.tile([128, 64 * CHUNK], F32, tag="ps")
        for _ in range(44):
            nc.tensor.matmul(
                wps[:, 0:512],
                lhsT=zeros[:, 0:128],
                rhs=zeros[:, 0:512],
                start=True,
                stop=True,
            )

        qts, kts, vas = [], [], []
        for s in range(HPC):
            qs = hpools[s].tile([64, S], F16, tag=f"qt{s}")
            ks = hpools[s].tile([64, S], F16, tag=f"kt{s}")
            vs = hpools[s].tile([128, 48 * 65], F16, tag=f"va{s}")
            # Head 0's tensors go on their own DMA queue (scalar's): the DMA
            # engines round-robin all queued descriptors, so on a shared
            # queue even the first tensors complete only near the end of the
            # whole ~5 MB load, idling the PE for ~7 us right when the HAM
            # warm-up window expires.
            eng = nc.scalar if s == 0 else nc.sync
            eng.dma_start(out=qs[:], in_=qt[s])
            eng.dma_start(out=ks[:], in_=kt[s])
            eng.dma_start(out=vs[:], in_=va[s])
            qts.append(qs)
            kts.append(ks)
            vas.append(vs)

        # Prefetch the exp ACT table set (~2.7us) while input DMAs run.
        # Emitted AFTER the dma_start submissions: the table load would
        # otherwise sit ahead of head 0's DMA submits on the scalar queue
        # and delay the input gate by its ~2.7us.
        scr = const.tile([128, 64], F16, tag="scr")
        nc.scalar.activation(
            out=scr[:],
            in_=zeros[:, 0:64],
            func=mybir.ActivationFunctionType.Exp,
            scale=0.125,
        )

        # Global chunk list across heads so the PE pipeline never drains.
        items = []  # (s, chunk_cols)
        for s in range(HPC):
            for ch in _chunks_of(schedules[s]["cols"]):
                items.append((s, ch))

        # psO bank state: one open (s, g) accumulation at a time on the PV
        # side; finalize (copy + DMA) when the next group begins.
        state = {"key": None, "tile": None}

        def finalize_group():
            if state["key"] is None:
                return
            s, g = state["key"]
            o_sb = outp.tile([65, 512], F16, tag="o")
            nc.vector.tensor_copy(out=o_sb[:], in_=state["tile"][0:65, :])
            nc.sync.dma_start(out=ot[s][:, 512 * g : 512 * (g + 1)], in_=o_sb[:])
            state["key"] = None
            state["tile"] = None

        def emit_pv(s, chunk, pt, plan):
            g = chunk[0]["g"]
            if state["key"] != (s, g):
                finalize_group()
                oT = psO.tile([128, 512], F32, tag="psO")
                state["key"] = (s, g)
                state["tile"] = oT
                # start=True on the bank's first PV clears has_written for
                # the WHOLE bank (HW-probed), so every element's first writer
                # overwrites stale data and later writers accumulate — no
                # zero-open matmul needed.
                state["first"] = True
            oT = state["tile"]
            for t, vi, i0, qb0, n in plan:
                w = vas[s][:, 65 * (3 * t + vi) : 65 * (3 * t + vi + 1)]
                q0 = qb0 - 8 * g
                nc.tensor.matmul(
                    oT[0:65, 64 * q0 : 64 * (q0 + n)],
                    lhsT=w,
                    rhs=pt[:, 64 * i0 : 64 * (i0 + n)],
                    start=state.pop("first", False),
                    stop=True,
                    skip_group_check=True,
                )

        pending = deque()
        for idx, (s, chunk) in enumerate(items):
            L = len(chunk)
            ps = psS.tile([128, 64 * CHUNK], F32, tag="ps")

            # QK: lhsT = K^T pair (fixed per t), rhs = Q^T qb-run. Runs split
            # at t changes, qb gaps and PSUM bank (512-col) edges.
            runs = []
            cur = [(0, chunk[0])]
            for i, c in enumerate(chunk[1:], start=1):
                pi, pc = cur[-1]
                if (
                    c["t"] == pc["t"]
                    and c["qb"] == pc["qb"] + 1
                    and (i // 8) == (cur[0][0] // 8)
                ):
                    cur.append((i, c))
                else:
                    runs.append(cur)
                    cur = [(i, c)]
            runs.append(cur)
            for run in runs:
                i0, rc = run[0]
                n = len(run)
                nc.tensor.matmul(
                    ps[:, 64 * i0 : 64 * (i0 + n)],
                    lhsT=kts[s][:, 128 * rc["t"] : 128 * (rc["t"] + 1)],
                    rhs=qts[s][:, 64 * rc["qb"] : 64 * (rc["qb"] + n)],
                    start=True,
                    stop=True,
                )

            pt = pts.tile([128, 64 * CHUNK], F16, tag="pt")
            nc.scalar.activation(
                out=pt[:, : 64 * L],
                in_=ps[:, : 64 * L],
                func=mybir.ActivationFunctionType.Exp,
                scale=0.125,
            )

            # Fixups (alternate the engine per chunk to split the load
            # between DVE and Pool): causal triangles on diagonal blocks,
            # then batched memsets of dead halves inside mixed PV runs.
            plan, need_top, need_bot = _plan_pv(chunk)
            eng_tri = nc.vector if idx % 2 == 0 else nc.gpsimd
            eng_ms = nc.gpsimd if idx % 2 == 0 else nc.vector
            for i, c in enumerate(chunk):
                if c["top"] and c["qb"] == c["kb1"]:
                    eng_tri.tensor_mul(
                        pt[0:64, 64 * i : 64 * (i + 1)],
                        pt[0:64, 64 * i : 64 * (i + 1)],
                        tri_t[0:64],
                    )
                if c["bot"] and c["qb"] == c["kb2"]:
                    eng_tri.tensor_mul(
                        pt[64:128, 64 * i : 64 * (i + 1)],
                        pt[64:128, 64 * i : 64 * (i + 1)],
                        tri_t[64:128],
                    )
            for half, need in ((slice(0, 64), need_top), (slice(64, 128), need_bot)):
                i = 0
                while i < L:
                    if need[i]:
                        j = i
                        while j + 1 < L and need[j + 1]:
                            j += 1
                        eng_ms.memset(pt[half, 64 * i : 64 * (j + 1)], 0.0)
                        i = j + 1
                    else:
                        i += 1

            pending.append((s, chunk, pt, plan))
            if len(pending) > LOOKAHEAD:
                emit_pv(*pending.popleft())
        while pending:
            emit_pv(*pending.popleft())
        finalize_group()

    _strip_repeated_weights(nc)
    _split_multi_waits(nc)
    return nc


def _prep_inputs(q, k, v, schedules):
    """Per-core input arrays keyed as the programs expect."""
    # flat head g = b*H + h
    qt_all = np.ascontiguousarray(
        q.transpose(0, 2, 3, 1).reshape(B * H, D, S).astype(np.float16)
    )
    kt_nat = k.transpose(0, 2, 3, 1).reshape(B * H, D, S).astype(np.float16)
    kt_nat = kt_nat.reshape(B * H, D, NB, 64)
    kt_all = np.empty_like(kt_nat)
    for g in range(B * H):
        order = [kb for p in schedules[g]["pairs"] for kb in p]
        kt_all[g] = kt_nat[g][:, order, :]
    kt_all = np.ascontiguousarray(kt_all.reshape(B * H, D, S))
    v_aug = np.concatenate([v, np.ones((B, S, H, 1), v.dtype)], axis=3)  # [B,S,H,65]
    vb_all = v_aug.transpose(0, 2, 1, 3).reshape(B * H, NB, 64, 65)  # [g, kb, 64, 65]
    # va[g]: per pair t, three variants (F=[V1;V2], T=[V1;0], B=[0;V2]);
    # rows 0:64 = V[kb1] block, rows 64:128 = V[kb2].
    va_all = np.zeros((B * H, 128, 48 * 65), np.float16)
    for g in range(B * H):
        for t, (kb1, kb2) in enumerate(schedules[g]["pairs"]):
            va_all[g, 0:64, 195 * t : 195 * t + 65] = vb_all[g, kb1]
            va_all[g, 64:128, 195 * t : 195 * t + 65] = vb_all[g, kb2]
            va_all[g, 0:64, 195 * t + 65 : 195 * t + 130] = vb_all[g, kb1]
            va_all[g, 64:128, 195 * t + 130 : 195 * t + 195] = vb_all[g, kb2]
    # tri[kl, ql] = 1 where kl <= ql (allowed), both halves
    triu = np.triu(np.ones((64, 64), np.float16))
    tri_full = np.ascontiguousarray(np.concatenate([triu, triu], axis=0))
    in_maps = []
    for c in range(N_CORES):
        sl = slice(HPC * c, HPC * (c + 1))
        in_maps.append(
            {
                "qt": qt_all[sl],
                "kt": kt_all[sl],
                "va": va_all[sl],
                "tri": tri_full,
            }
        )
    return in_maps


def _schedules(block_mask):
    """Per flat head: greedy k-block pairing + column schedule."""
    masks_all = np.asarray(block_mask).reshape(B * H, NB, NB)
    scheds = []
    for g in range(B * H):
        pairs = _match_pairs(masks_all[g])
        scheds.append({"pairs": pairs, "cols": _head_schedule(masks_all[g], pairs)})
    return scheds


_PROG_CACHE = {}


def _get_programs(block_mask, schedules):
    key = np.asarray(block_mask).tobytes()
    if key not in _PROG_CACHE:
        _PROG_CACHE[key] = [
            build_program(schedules[HPC * c : HPC * (c + 1)]) for c in range(N_CORES)
        ]
    return _PROG_CACHE[key]


def run_cores(ncs, in_maps, trace=False):
    """Run the 8 per-core programs concurrently on the 8 devices."""
    import jax

    devs = jax.devices()
    results = [None] * N_CORES
    errs = [None] * N_CORES

    def _run(c):
        try:
            with jax.default_device(devs[c]):
                r = run_bass_kernel_spmd(
                    ncs[c], [in_maps[c]], core_ids=[0], trace=trace and c == 0
                )
                results[c] = r
        except Exception as e:  # noqa: BLE001
            errs[c] = e

    threads = [threading.Thread(target=_run, args=(c,)) for c in range(N_CORES)]
    for t in threads:
        t.start()
    for t in threads:
        t.join()
    for c, e in enumerate(errs):
        if e is not None:
            raise RuntimeError(f"core {c} failed") from e
    return results


def kernel(q, k, v, block_mask):
    q = np.asarray(q, dtype=np.float32)
    k = np.asarray(k, dtype=np.float32)
    v = np.asarray(v, dtype=np.float32)
    block_mask = np.asarray(block_mask).astype(bool)

    schedules = _schedules(block_mask)
    in_maps = _prep_inputs(q, k, v, schedules)
    ncs = _get_programs(block_mask, schedules)
    results = run_cores(ncs, in_maps)

    out = np.empty((B, S, H, D), np.float32)
    for c in range(N_CORES):
        ot = results[c].results[0]["ot"]  # [HPC, 65, S] fp16
        for s in range(HPC):
            g = HPC * c + s
            b, h = divmod(g, H)
            o_un = ot[s, :D, :].astype(np.float32)  # [D, S] unnormalized
            l = ot[s, D, :].astype(np.float32)  # [S]
            out[b, :, h, :] = (o_un / l[None, :]).T
    return out
